# revision 10
# baseline (speedup 1.0000x reference)
"""CountSketch kernel for Trainium2 (8 NeuronCores, SPMD data-parallel).

out[b, i_hash[j]] += x[b, j] * s_hash[j]
  x: [4096, 16384] f32, s_hash: [16384] f32, i_hash: [16384] int64 -> out [4096, 1024] f32

Strategy (batch-sharded, device-side scatter):
  - shard x by batch across 8 cores (512 rows each), host supplies each
    core its shard transposed (xT [16384, 512], a pure layout change).
  - host computes (from the tiny i_hash/s_hash vectors only) a
    bucket-sorted column order `perm`, banded one-hot +/-1 weight blocks R
    (signs folded in), and int16 gather indices.
  - each core: gpsimd.dma_gather pulls rows of xT in bucket-sorted order
    (2KB descriptors) into SBUF tiles [128, slots, 512]; each 128-row
    sorted chunk multiplies a small [128, M] weight block on the Tensor
    engine, accumulating out^T = [1024 f, 512 b] across all 128 chunks
    directly in PSUM (8 banks x [128, 512] = exactly all of PSUM).
  - PSUM banks are copied out once at the end -> outT [1024, 512] in DRAM.
  - host transposes/concatenates the 8 outT shards into [4096, 1024].
"""
import numpy as np
from contextlib import ExitStack

import concourse.bacc as bacc
import concourse.tile as tile
from concourse import mybir
from concourse import bass_utils

D_IN = 16384
D_F = 1024
B = 4096
NCORES = 8
BSH = B // NCORES          # 512 batch rows per core
CHUNK = 128                # sorted rows per matmul chunk
N_CHUNKS = D_IN // CHUNK   # 128
GROUP = 1024               # indices per dma_gather call (ring limit < 2048 descs)
SLOTS = GROUP // CHUNK     # 16
NG = D_IN // GROUP         # 8

F32 = mybir.dt.float32
F32R = mybir.dt.float32r
I16 = mybir.dt.int16

MM_DTYPE = F32R            # tensor-engine stream dtype (f32r = full-rate fp32)


def _build_metadata(i_hash: np.ndarray, s_hash: np.ndarray):
    """Sort columns by bucket; build per-chunk banded weight blocks.

    Returns (perm, idx_tile, r_all, mm_descs) where mm_descs is a list of
    (chunk, bank, p0, M, col_offset) and r_all is the packed [128, total]
    f32 weight matrix (columns: 128 zeros first, then each block).
    """
    i_hash = np.asarray(i_hash).astype(np.int64).ravel()
    s_hash = np.asarray(s_hash).astype(np.float32).ravel()
    perm = np.argsort(i_hash, kind="stable")
    f_sorted = i_hash[perm]
    s_sorted = s_hash[perm]

    blocks = [np.zeros((CHUNK, CHUNK), np.float32)]  # zero block @ col 0
    off = CHUNK
    mm_descs = []
    for c in range(N_CHUNKS):
        fs = f_sorted[c * CHUNK:(c + 1) * CHUNK]
        ss = s_sorted[c * CHUNK:(c + 1) * CHUNK]
        for h in np.unique(fs // 128):
            # f32r matmuls require the full 128-wide col group (M=128, p0=0);
            # fp32 col tiling is silently wrong on HW, so R covers the bank.
            sel = (fs // 128) == h
            fl = (fs[sel] - h * 128).astype(np.int64)  # local f in [0,128)
            R = np.zeros((CHUNK, CHUNK), np.float32)
            rows = np.nonzero(sel)[0]
            R[rows, fl] = ss[sel]
            blocks.append(R)
            mm_descs.append((c, int(h), 0, CHUNK, off))
            off += CHUNK
    r_all = np.concatenate(blocks, axis=1)

    # int16 gather indices, wrapped in 16 partitions, replicated to 128.
    idx16 = np.empty((16, D_IN // 16), np.int16)
    for p in range(16):
        idx16[p, :] = perm[p::16]
    idx_tile = np.tile(idx16, (8, 1))
    return perm, idx_tile, r_all, mm_descs


def _build_bass(mm_descs, total_w):
    nc = bacc.Bacc("TRN2", target_bir_lowering=False, debug=False, num_devices=1)
    xT = nc.dram_tensor("xT", [D_IN, BSH], MM_DTYPE, kind="ExternalInput").ap()
    rw = nc.dram_tensor("rw", [CHUNK, total_w], MM_DTYPE, kind="ExternalInput").ap()
    idx = nc.dram_tensor("idx", [CHUNK, D_IN // 16], I16, kind="ExternalInput").ap()
    outT = nc.dram_tensor("outT", [D_F, BSH], F32, kind="ExternalOutput").ap()

    by_chunk = {}
    for (c, h, p0, M, off) in mm_descs:
        by_chunk.setdefault(c, []).append((h, p0, M, off))

    with tile.TileContext(nc) as tc, ExitStack() as ctx:
        wpool = ctx.enter_context(tc.tile_pool(name="w", bufs=1))
        xpool = ctx.enter_context(tc.tile_pool(name="x", bufs=3))
        opool = ctx.enter_context(tc.tile_pool(name="o", bufs=2))
        ppool = ctx.enter_context(tc.tile_pool(name="ps", bufs=1, space="PSUM"))

        wt = wpool.tile([CHUNK, total_w], MM_DTYPE, name="wt")
        nc.sync.dma_start(wt[:], rw[:])
        it = wpool.tile([CHUNK, D_IN // 16], I16, name="it")
        nc.sync.dma_start(it[:], idx[:])

        psums = [ppool.tile([128, BSH], F32, name=f"psum{h}", tag=f"psum{h}")
                 for h in range(8)]

        # Zero all 8 banks: matmul with the zero weight block (start=True).
        for h in range(8):
            nc.tensor.matmul(
                psums[h][:, :],
                lhsT=wt[:, 0:CHUNK],
                rhs=wt[:, 0:BSH],
                start=True, stop=False,
            )

        for g in range(NG):
            xt = xpool.tile([128, SLOTS, BSH], MM_DTYPE, name="xt")
            nc.gpsimd.dma_gather(
                out_ap=xt[:],
                in_ap=xT[:],
                idxs_ap=it[:, g * (GROUP // 16):(g + 1) * (GROUP // 16)],
                num_idxs=GROUP,
                num_idxs_reg=GROUP,
                elem_size=BSH,
            )
            for s in range(SLOTS):
                c = g * SLOTS + s
                rhs = xt[:, s, :]
                for (h, p0, M, off) in by_chunk.get(c, []):
                    nc.tensor.matmul(
                        psums[h][p0:p0 + M, :],
                        lhsT=wt[:, off:off + M],
                        rhs=rhs,
                        start=False, stop=False,
                    )

        # Close each bank's accumulation group with a full-width zero matmul
        # (stop only clears sim group flags for the partitions it covers).
        for h in range(8):
            nc.tensor.matmul(
                psums[h][:, :],
                lhsT=wt[:, 0:CHUNK],
                rhs=wt[:, 0:BSH],
                start=False, stop=True,
            )

        for h in range(8):
            ot = opool.tile([128, BSH], F32, name="ot")
            nc.scalar.copy(ot[:], psums[h][:])
            nc.sync.dma_start(outT[128 * h:128 * (h + 1), :], ot[:])

    nc.compile()
    return nc


_CACHE = {}
_LAST_RESULTS = None


def _get_compiled(i_hash, s_hash):
    key = (i_hash.tobytes(), s_hash.tobytes())
    if key not in _CACHE:
        perm, idx_tile, r_all, mm_descs = _build_metadata(i_hash, s_hash)
        nc = _build_bass(mm_descs, r_all.shape[1])
        _CACHE[key] = (nc, idx_tile, r_all)
    return _CACHE[key]


def predicted_ns():
    """Cost-model (TimelineSim) predicted single-core execution time in ns."""
    if not _CACHE:
        return None
    nc = next(iter(_CACHE.values()))[0]
    from concourse.timeline_sim import TimelineSim
    return int(TimelineSim(nc).simulate())


def kernel(x, s_hash, i_hash):
    x = np.asarray(x)
    in_dtype = x.dtype
    x = np.ascontiguousarray(x, dtype=np.float32)
    i_hash = np.asarray(i_hash).astype(np.int64).ravel()
    s_hash = np.asarray(s_hash).astype(np.float32).ravel()

    nc, idx_tile, r_all = _get_compiled(i_hash, s_hash)

    xt_full = x.T  # [16384, 4096] view
    in_maps = []
    for k in range(NCORES):
        xT_k = np.ascontiguousarray(xt_full[:, k * BSH:(k + 1) * BSH])
        in_maps.append({"xT": xT_k, "rw": r_all, "idx": idx_tile})

    res = bass_utils.run_bass_kernel_spmd(nc, in_maps, core_ids=list(range(NCORES)))
    global _LAST_RESULTS
    _LAST_RESULTS = res
    out = np.concatenate(
        [np.ascontiguousarray(res.results[k]["outT"].T) for k in range(NCORES)],
        axis=0,
    )
    return out.astype(in_dtype, copy=False)
